# revision 20
# baseline (speedup 1.0000x reference)
"""NetTGCN forward pass on 8 Trainium2 NeuronCores (Bass/Tile).

Key algorithmic move: the reference's real(FFT) along the 30 time taps is a
rank-16 linear map (cos(2*pi*t*f/30) has identical columns for f and 30-f),
so layer 1's Chebyshev recurrence runs on 16 frequency channels per batch
instead of 30 taps - half the spmv FLOPs of a direct fold.

Sharding:
  Layer 1 (4096-node graph): 2-way node-shard x 4-way batch-shard. Per core:
  8 batches x 16 freqs = 128 channels, 2048 own nodes. The state is kept
  CHANNEL-major [128 c, 2048 n]; the spmv is out = state_blk.T @ M^T-rows
  (stationary = node-major state blocks from the gathered DRAM copy, moving =
  SBUF-resident M^T shard, N=512), which directly produces the channel-major
  next state, so the per-k W-contraction needs no transposes. The per-step
  exchange is a 2-rank AllGather (pairs (c, c+4)) of the XBAR-DMA-transposed
  fp16 state (0.5 MB wire, ~16 us), hidden under the other Chebyshev chain's
  spmv (even/odd chains via M = 4*A'^2). fp16 everywhere in layer 1 (states
  included): simulated end-to-end error 1.8e-3.
  Core (h, q) = core h*4+q owns node half h and batches b_loc -> global
  batch 4*b_loc + q; L2 core j owns batches 4j..4j+3.
  Layer 2 (1024-node graph): batch-parallel (core j handles batches
  4j..4j+3 after an 8-rank AllToAll), zero collectives in the loop,
  same channel-major spmv structure, A2 resident, fc1w prefetched meanwhile.
  Head: h2 features redistributed with an 8-rank AllToAll so fc1 is sharded
  over its 65536-row contraction; partial z AllReduced; fc2 + log_softmax
  computed redundantly on every core. Host un-permutes the 32 rows.
"""

import sys

if "/opt/trn_rl_repo" not in sys.path:
    sys.path.insert(0, "/opt/trn_rl_repo")

import numpy as np

import concourse.bacc as bacc
import concourse.mybir as mybir
import concourse.bass_utils as _bu
from concourse.bass_utils import run_bass_kernel_spmd
from concourse.tile import TileContext
from concourse.masks import make_identity

_bu.upload_artifacts = lambda tmpdir: f"file://{tmpdir}"  # no bucket in sandbox

F16 = mybir.dt.float16
F32 = mybir.dt.float32
AX = mybir.AxisListType
ALU = mybir.AluOpType
ACT = mybir.ActivationFunctionType

B, N0, T, K = 32, 4096, 30, 25
G1, G2, D, C = 32, 64, 512, 10
N2 = N0 // 4
NF = 16                 # rank of the real-FFT cosine map
NCORES = 8
NH = N0 // 2            # 2048 own nodes per core (node half)
P2H = N2 // 2           # 512 own pooled nodes
FBLK = (N2 * G2) // NCORES  # 8192 fc1 contraction rows per core

GPAIR = [[0, 4], [1, 5], [2, 6], [3, 7]]
G8 = [list(range(NCORES))]


def _f16(a):
    return np.ascontiguousarray(np.asarray(a, np.float32).astype(np.float16))


def _dense_adj(edge_index, n):
    row = edge_index[0].astype(np.int64)
    col = edge_index[1].astype(np.int64)
    deg = np.zeros(n, np.float32)
    np.add.at(deg, row, 1.0)
    dis = np.where(deg > 0, 1.0 / np.sqrt(np.maximum(deg, 1.0)), 0.0).astype(np.float32)
    w = (-dis[row] * dis[col]).astype(np.float32)
    a = np.zeros((n, n), np.float32)
    np.add.at(a, (row, col), w)
    return a


def build_program(dbg=False):
    nc = bacc.Bacc("TRN2", target_bir_lowering=False, debug=False,
                   num_devices=NCORES)

    x_cm_in = nc.dram_tensor("x_cm", [128, 2 * NH], F16, kind="ExternalInput")
    c16_in = nc.dram_tensor("c16w", [128, 2 * 128], F16, kind="ExternalInput")
    m1t_in = nc.dram_tensor("m1t", [N0, NH], F16, kind="ExternalInput")
    a1t_in = nc.dram_tensor("a1t", [N0, NH], F16, kind="ExternalInput")
    a2t_in = nc.dram_tensor("a2t", [N2, N2], F16, kind="ExternalInput")
    m2t_in = nc.dram_tensor("m2t", [N2, N2], F16, kind="ExternalInput")
    w1_in = nc.dram_tensor("w1a", [128, K * 2 * 128], F16, kind="ExternalInput")
    w2_in = nc.dram_tensor("w2a", [128, K * 2 * 128], F16, kind="ExternalInput")
    b1_in = nc.dram_tensor("b1v", [128, 1], F32, kind="ExternalInput")
    b2_in = nc.dram_tensor("b2v", [128, 1], F32, kind="ExternalInput")
    fc1w_in = nc.dram_tensor("fc1w", [FBLK, D], F16, kind="ExternalInput")
    fc1b_in = nc.dram_tensor("fc1b", [B, D], F32, kind="ExternalInput")
    fc2w_in = nc.dram_tensor("fc2w", [D, C], F16, kind="ExternalInput")
    fc2b_in = nc.dram_tensor("fc2b", [B, C], F32, kind="ExternalInput")

    out_t = nc.dram_tensor("out", [B, C], F32, kind="ExternalOutput")
    if dbg:
        h1_dbg = nc.dram_tensor("h1_dbg", [256, NH], F32, kind="ExternalOutput")
        l2i_dbg = nc.dram_tensor("l2i_dbg", [128, N2], F32, kind="ExternalOutput")
        h2_dbg = nc.dram_tensor("h2_dbg", [256, N2], F32, kind="ExternalOutput")
        ccpo_dbg = nc.dram_tensor("ccpo_dbg", [256, P2H], F16,
                                  kind="ExternalOutput")
        nm0_dbg = nc.dram_tensor("nm0_dbg", [N2, 128], F16,
                                 kind="ExternalOutput")
        t22_dbg = nc.dram_tensor("t22_dbg", [128, N2], F32,
                                 kind="ExternalOutput")
        z_dbg = nc.dram_tensor("z_dbg", [B, D], F32, kind="ExternalOutput")

    cc1_in = [nc.dram_tensor(f"cc1i{i}", [NH, 128], F16) for i in range(2)]
    cc1_out = [nc.dram_tensor(f"cc1o{i}", [N0, 128], F16) for i in range(2)]
    ccp_in = nc.dram_tensor("ccp_in", [256, P2H], F16)
    ccp_out = nc.dram_tensor("ccp_out", [256, P2H], F16)
    cch_in = [nc.dram_tensor(f"cchi{i}", [NCORES * 4, FBLK // 2], F16)
              for i in range(2)]
    cch_out = [nc.dram_tensor(f"ccho{i}", [NCORES * 4, FBLK // 2], F16)
               for i in range(2)]
    ccz_in = [nc.dram_tensor(f"cczi{i}", [B, D], F32) for i in range(2)]
    ccz_out = [nc.dram_tensor(f"cczo{i}", [B, D], F32, addr_space="Shared")
               for i in range(2)]

    with TileContext(nc) as tc:
        # ======================= LAYER 1 =======================
        with tc.tile_pool(name="l1c", bufs=1) as l1c, \
             tc.tile_pool(name="l1mv", bufs=1) as l1mv, \
             tc.tile_pool(name="l1st", bufs=5) as l1st, \
             tc.tile_pool(name="l1g", bufs=3) as l1g, \
             tc.tile_pool(name="l1nm", bufs=2) as l1nm, \
             tc.tile_pool(name="ps_y", bufs=3, space="PSUM") as ps_y, \
             tc.tile_pool(name="ps_ct", bufs=2, space="PSUM") as ps_ct:

            w1a = l1c.tile([128, K, 2, 128], F16)
            nc.sync.dma_start(
                w1a[:], w1_in.ap().rearrange("p (k b c) -> p k b c", k=K, b=2))
            b1v = l1c.tile([128, 1], F32)
            nc.sync.dma_start(b1v[:], b1_in.ap())
            h1_sb = l1c.tile([128, 2, NH], F32)
            nc.any.memset(h1_sb[:], 0.0)

            # moving-operand buffer: holds a1t for k=1, then m1t for k>=2.
            # Bulk loads ride the scalar-engine HWDGE queue so the
            # latency-critical sync-queue DMAs are not stuck behind them.
            mv = l1mv.tile([128, 32, NH], F16)
            a1_v = a1t_in.ap().rearrange("(t p) n -> t p n", p=128)
            m1_v = m1t_in.ap().rearrange("(t p) n -> t p n", p=128)
            for mb in range(32):
                nc.scalar.dma_start(mv[:, mb, :], a1_v[mb])

            tx = {}

            def xbar_ag(k):
                nm = l1nm.tile([128, NH // 128, 128], F16, tag="nm",
                               name=f"nm{k}")
                nc.sync.dma_start_transpose(nm[:], tx[k][:])
                cin, cout = cc1_in[k % 2], cc1_out[k % 2]
                nc.sync.dma_start(
                    cin.ap().rearrange("(t p) c -> p t c", p=128), nm[:])
                nc.gpsimd.collective_compute(
                    "AllGather", ALU.bypass, replica_groups=GPAIR,
                    ins=[cin.ap()], outs=[cout.ap()])

            def contract(k):
                for bb in range(2):
                    for ns in range(4):
                        cps = ps_ct.tile([128, 512], F32, tag="ct",
                                         name=f"ct{k}_{bb}_{ns}")
                        nc.tensor.matmul(cps[:], w1a[:, k, bb, :],
                                         tx[k][:, 512 * ns:512 * (ns + 1)],
                                         start=True, stop=True)
                        nc.vector.tensor_tensor(
                            h1_sb[:, bb, 512 * ns:512 * (ns + 1)],
                            h1_sb[:, bb, 512 * ns:512 * (ns + 1)],
                            cps[:], ALU.add)

            # ---- x' = x @ C16 (channel-major) ----
            with tc.tile_pool(name="l1x", bufs=1) as l1x:
                c16 = l1x.tile([128, 2, 128], F16)
                nc.sync.dma_start(
                    c16[:], c16_in.ap().rearrange("p (b c) -> p b c", b=2))
                x_v = x_cm_in.ap().rearrange("p (b n) -> p b n", b=2)
                tx[0] = l1st.tile([128, NH], F16, tag="tx", name="tx0")
                for bb2 in range(2):
                    xh = l1x.tile([128, NH], F16, tag="xh", name=f"xh{bb2}")
                    nc.sync.dma_start(xh[:], x_v[:, bb2, :])
                    for ns in range(4):
                        xps = ps_ct.tile([128, 512], F32, tag="ct",
                                         name=f"xp{bb2}_{ns}")
                        nc.tensor.matmul(xps[:], c16[:, bb2, :],
                                         xh[:, 512 * ns:512 * (ns + 1)],
                                         start=True, stop=True)
                        o = tx[0][:, 512 * ns:512 * (ns + 1)]
                        if bb2 == 0:
                            nc.vector.tensor_copy(o, xps[:])
                        else:
                            nc.vector.tensor_tensor(o, o, xps[:], ALU.add)
                xbar_ag(0)

            # ---- Chebyshev steps; contract(k-1) emitted between spmvs ----
            for k in range(1, K):
                gi = 0 if k == 1 else k % 2   # k=1 consumes the x' gather
                gsrc = cc1_out[gi].ap().rearrange("(t p) c -> p t c", p=128)
                tx[k] = l1st.tile([128, NH], F16, tag="tx", name=f"tx{k}")
                stt = []
                for hb in range(2):
                    s = l1g.tile([128, 16, 128], F16, tag="g",
                                 name=f"g{k}_{hb}")
                    nc.sync.dma_start(s[:], gsrc[:, 16 * hb:16 * (hb + 1), :])
                    stt.append(s)
                for half in range(2):
                    yp = ps_y.tile([128, 2, 512], F32, tag="y",
                                   name=f"y{k}_{half}")
                    for mb in range(32):
                        for j in range(2):
                            nc.tensor.matmul(
                                yp[:, j, :], stt[mb // 16][:, mb % 16, :],
                                mv[:, mb, 1024 * half + 512 * j:
                                   1024 * half + 512 * (j + 1)],
                                start=(mb == 0), stop=(mb == 31))
                    o = tx[k][:, 1024 * half:1024 * (half + 1)]
                    ypf = yp[:].rearrange("p a b -> p (a b)")
                    if k == 1:
                        nc.vector.tensor_scalar_mul(o, ypf, 0.5)
                    elif k == 2:
                        nc.vector.tensor_scalar_mul(o, ypf, 0.5)
                        nc.vector.tensor_tensor(
                            o, o, tx[0][:, 1024 * half:1024 * (half + 1)],
                            ALU.subtract)
                    elif k == 3:
                        p1 = tx[1][:, 1024 * half:1024 * (half + 1)]
                        nc.vector.tensor_tensor(o, ypf, p1, ALU.subtract)
                        nc.vector.tensor_tensor(o, o, p1, ALU.subtract)
                        nc.vector.tensor_tensor(o, o, p1, ALU.subtract)
                    else:
                        p2 = tx[k - 2][:, 1024 * half:1024 * (half + 1)]
                        p4 = tx[k - 4][:, 1024 * half:1024 * (half + 1)]
                        nc.vector.tensor_tensor(o, ypf, p2, ALU.subtract)
                        nc.vector.tensor_tensor(o, o, p2, ALU.subtract)
                        nc.vector.tensor_tensor(o, o, p4, ALU.subtract)
                if k == 1:
                    # refill the moving buffer with m1t now that a1t is done
                    for mb in range(32):
                        nc.scalar.dma_start(mv[:, mb, :], m1_v[mb])
                if k < K - 2:
                    xbar_ag(k)
                contract(k - 1)
                tx.pop(k - 4, None)
            contract(K - 1)

            # ---- bias + relu + maxpool4 along nodes ----
            h1p = l1c.tile([128, 2, P2H], F16)
            for bb in range(2):
                nc.scalar.activation(h1_sb[:, bb, :], h1_sb[:, bb, :],
                                     ACT.Relu, bias=b1v[:])
                h4 = h1_sb[:, bb, :].rearrange("p (n f) -> p n f", f=4)
                nc.vector.tensor_tensor(h1p[:, bb, :], h4[:, :, 0],
                                        h4[:, :, 1], ALU.max)
                nc.vector.tensor_tensor(h1p[:, bb, :], h1p[:, bb, :],
                                        h4[:, :, 2], ALU.max)
                nc.vector.tensor_tensor(h1p[:, bb, :], h1p[:, bb, :],
                                        h4[:, :, 3], ALU.max)
            if dbg:
                nc.sync.dma_start(
                    h1_dbg.ap().rearrange("(b p) n -> p b n", p=128), h1_sb[:])

            # 8-rank AllToAll of pooled features. Batch ownership is chosen so
            # slot j (rows 32j..32j+32 = b_loc j's g-rows x own 512 nodes) is
            # exactly what L2 core j needs from this core; the output blocks
            # are then read rank-uniformly.
            nc.sync.dma_start(
                ccp_in.ap().rearrange("(b p) c -> p b c", p=128), h1p[:])
            nc.gpsimd.collective_compute(
                "AllToAll", ALU.bypass, replica_groups=G8,
                ins=[ccp_in.ap()], outs=[ccp_out.ap()])

        # ======================= LAYER 2 =======================
        with tc.tile_pool(name="l2c", bufs=1) as l2c, \
             tc.tile_pool(name="l2st", bufs=5) as l2st, \
             tc.tile_pool(name="l2nm", bufs=3) as l2nm:

            a2t = l2c.tile([128, N2 // 128, N2], F16)
            nc.scalar.dma_start(
                a2t[:], a2t_in.ap().rearrange("(t p) n -> p t n", p=128))
            m2t = l2c.tile([128, N2 // 128, N2], F16)
            nc.scalar.dma_start(
                m2t[:], m2t_in.ap().rearrange("(t p) n -> p t n", p=128))
            w2a = l2c.tile([128, K, 2, 128], F16)
            nc.scalar.dma_start(
                w2a[:], w2_in.ap().rearrange("p (k h c) -> p k h c", k=K, h=2))
            b2v = l2c.tile([128, 1], F32)
            nc.sync.dma_start(b2v[:], b2_in.ap())
            # preload fc1w for the head while layer 2 computes
            fc1w = l2c.tile([128, FBLK // 128, D], F16)
            nc.scalar.dma_start(
                fc1w[:], fc1w_in.ap().rearrange("(t p) d -> p t d", p=128))
            h2_sb = l2c.tile([128, 2, N2], F32)
            nc.any.memset(h2_sb[:], 0.0)

            ident2 = l2c.tile([128, 128], F16)
            make_identity(nc, ident2[:])
            with tc.tile_pool(name="ps2_y", bufs=2, space="PSUM") as ps2_y, \
                 tc.tile_pool(name="ps2_ct", bufs=2, space="PSUM") as ps2_ct, \
                 tc.tile_pool(name="ps2_tr", bufs=2, space="PSUM") as ps2_tr:

                tx2 = {}
                nm2 = {}
                # out block r=(h', q') = core r's slot for me: batch 4*my_j+q'
                # (g1-rows) x n2-half h'
                tx2[0] = l2st.tile([128, N2], F16, tag="tx2", name="tx20")
                for hp in range(2):
                    for qp in range(4):
                        nc.sync.dma_start(
                            tx2[0][32 * qp:32 * (qp + 1),
                                   512 * hp:512 * (hp + 1)],
                            ccp_out.ap()[32 * (4 * hp + qp):
                                         32 * (4 * hp + qp + 1), :])
                if dbg:
                    l2i = l2c.tile([128, N2], F32)
                    nc.vector.tensor_copy(l2i[:], tx2[0][:])
                    nc.sync.dma_start(l2i_dbg.ap(), l2i[:])
                    ccst = l2c.tile([128, 2, P2H], F16)
                    nc.sync.dma_start(
                        ccst[:],
                        ccp_out.ap().rearrange("(a p) c -> p a c", p=128))
                    nc.sync.dma_start(
                        ccpo_dbg.ap().rearrange("(a p) c -> p a c", p=128),
                        ccst[:])

                def xbar2(k):
                    # PE transposes (XBAR->PE edges proved racy on HW)
                    nm2[k] = l2nm.tile([128, N2 // 128, 128], F16, tag="nm2",
                                       name=f"nm2_{k}")
                    for g4 in range(2):
                        trp = ps2_tr.tile([128, 4, 128], F16, tag="tr2",
                                          name=f"tr2_{k}_{g4}")
                        for t in range(4):
                            mb = 4 * g4 + t
                            nc.tensor.transpose(
                                trp[:, t, :],
                                tx2[k][:, 128 * mb:128 * (mb + 1)],
                                ident2[:])
                            nc.any.tensor_copy(out=nm2[k][:, mb, :],
                                               in_=trp[:, t, :])

                def contract2(k):
                    for hh in range(2):
                        for ns in range(2):
                            cps = ps2_ct.tile([128, 512], F32, tag="ct2",
                                              name=f"c2_{k}_{hh}_{ns}")
                            nc.tensor.matmul(
                                cps[:], w2a[:, k, hh, :],
                                tx2[k][:, 512 * ns:512 * (ns + 1)],
                                start=True, stop=True)
                            nc.vector.tensor_tensor(
                                h2_sb[:, hh, 512 * ns:512 * (ns + 1)],
                                h2_sb[:, hh, 512 * ns:512 * (ns + 1)],
                                cps[:], ALU.add)

                # even/odd chains via M2 = 4*A2'^2 (same scheme as layer 1):
                # spmv k consumes nm2[k-2], so the update/XBAR latency of a
                # step hides under the other chain's spmv.
                xbar2(0)
                for k in range(1, K):
                    tx2[k] = l2st.tile([128, N2], F16, tag="tx2",
                                       name=f"tx2{k}")
                    src_nm = nm2[0] if k <= 2 else nm2[k - 2]
                    mvop = a2t if k == 1 else m2t
                    yp = ps2_y.tile([128, 2, 512], F32, tag="y2",
                                    name=f"y2_{k}")
                    for mb in range(N2 // 128):
                        for j in range(2):
                            nc.tensor.matmul(
                                yp[:, j, :], src_nm[:, mb, :],
                                mvop[:, mb, 512 * j:512 * (j + 1)],
                                start=(mb == 0), stop=(mb == N2 // 128 - 1))
                    ypf = yp[:].rearrange("p a b -> p (a b)")
                    if k == 1:
                        nc.vector.tensor_scalar_mul(tx2[1][:], ypf, 0.5)
                    elif k == 2:
                        nc.vector.tensor_scalar_mul(tx2[2][:], ypf, 0.5)
                        nc.vector.tensor_tensor(tx2[2][:], tx2[2][:],
                                                tx2[0][:], ALU.subtract)
                    elif k == 3:
                        nc.vector.tensor_tensor(tx2[3][:], ypf, tx2[1][:],
                                                ALU.subtract)
                        nc.vector.tensor_tensor(tx2[3][:], tx2[3][:],
                                                tx2[1][:], ALU.subtract)
                        nc.vector.tensor_tensor(tx2[3][:], tx2[3][:],
                                                tx2[1][:], ALU.subtract)
                    else:
                        nc.vector.tensor_tensor(tx2[k][:], ypf,
                                                tx2[k - 2][:], ALU.subtract)
                        nc.vector.tensor_tensor(tx2[k][:], tx2[k][:],
                                                tx2[k - 2][:], ALU.subtract)
                        nc.vector.tensor_tensor(tx2[k][:], tx2[k][:],
                                                tx2[k - 4][:], ALU.subtract)
                    if k < K - 2:
                        xbar2(k)
                    contract2(k - 1)
                    if dbg and k == 2:
                        t22 = l2c.tile([128, N2], F32, name="t22")
                        nc.vector.tensor_copy(t22[:], tx2[2][:])
                        nc.sync.dma_start(t22_dbg.ap(), t22[:])
                        nc.sync.dma_start(
                            nm0_dbg.ap().rearrange("(t p) c -> p t c", p=128),
                            nm2[0][:])
                    nm2.pop(k - 4, None)
                    tx2.pop(k - 4, None)
                contract2(K - 1)

                # bias + relu -> fp16 channel-major h2
                h2r = l2c.tile([128, 2, N2], F16)
                for hh in range(2):
                    nc.scalar.activation(h2r[:, hh, :], h2_sb[:, hh, :],
                                         ACT.Relu, bias=b2v[:])
                if dbg:
                    h2f = l2c.tile([128, 2, N2], F32)
                    nc.vector.tensor_copy(h2f[:], h2r[:])
                    nc.sync.dma_start(
                        h2_dbg.ap().rearrange("(h p) n -> p h n", p=128),
                        h2f[:])

            # =================== HEAD ===================
            with tc.tile_pool(name="hd", bufs=1) as hd, \
                 tc.tile_pool(name="hdt", bufs=2) as hdt, \
                 tc.tile_pool(name="ps3", bufs=2, space="PSUM") as ps3, \
                 tc.tile_pool(name="ps3z", bufs=1, space="PSUM") as ps3z:

                ident = hd.tile([128, 128], F16)
                make_identity(nc, ident[:])
                identf = hd.tile([32, 32], F32)
                make_identity(nc, identf[:])
                # ft[n2p, nt, (b4, g64)] fp16 via PE transposes
                ft = hd.tile([128, N2 // 128, 256], F16)
                for hh in range(2):
                    for nt in range(N2 // 128):
                        tmp = ps3.tile([128, 128], F16, tag="zt",
                                       name=f"t3_{hh}_{nt}")
                        nc.tensor.transpose(
                            tmp[:], h2r[:, hh, 128 * nt:128 * (nt + 1)],
                            ident[:])
                        for blh in range(2):
                            nc.any.tensor_copy(
                                out=ft[:, nt,
                                       64 * (2 * hh + blh):
                                       64 * (2 * hh + blh + 1)],
                                in_=tmp[:, 64 * blh:64 * (blh + 1)])
                # cch half hf: my partition-half hf of each n2 block ->
                # contiguous dest kt range [32*hf, 32*(hf+1)). Each half gets
                # its own AllToAll + fc1 half-contraction + AllReduce so the
                # collectives overlap the matmuls.
                for hf in range(2):
                    v = cch_in[hf].ap().rearrange("(r b) (p g) -> r p b g",
                                                  b=4, p=64)
                    for r in range(N2 // 128):
                        nc.sync.dma_start(
                            v[r],
                            ft[64 * hf:64 * (hf + 1), r, :].rearrange(
                                "p (b g) -> p b g", b=4))
                    nc.gpsimd.collective_compute(
                        "AllToAll", ALU.bypass, replica_groups=G8,
                        ins=[cch_in[hf].ap()], outs=[cch_out[hf].ap()])
                zb = hd.tile([32, D], F32)
                nc.sync.dma_start(zb[:], fc1b_in.ap())
                zar = []
                for hf in range(2):
                    flt_raw = hd.tile([128, FBLK // 256, B], F16, tag="fltr",
                                      name=f"fltr{hf}")
                    nc.sync.dma_start_transpose(flt_raw[:], cch_out[hf].ap())
                    flt = hd.tile([128, FBLK // 256, B], F16, tag="flt",
                                  name=f"flt{hf}")
                    nc.sync.dma_start(flt[:], flt_raw[:])
                    zps = ps3z.tile([32, D], F32, tag="zps", name=f"zps{hf}")
                    for kt in range(FBLK // 256):
                        nc.tensor.matmul(zps[:], flt[:, kt, :],
                                         fc1w[:, 32 * hf + kt, :],
                                         start=(kt == 0),
                                         stop=(kt == FBLK // 256 - 1))
                    zblk = hd.tile([32, D], F32, tag="zblk", name=f"zblk{hf}")
                    nc.vector.tensor_copy(zblk[:], zps[:])
                    nc.sync.dma_start(ccz_in[hf].ap(), zblk[:])
                    nc.gpsimd.collective_compute(
                        "AllReduce", ALU.add, replica_groups=G8,
                        ins=[ccz_in[hf].ap()], outs=[ccz_out[hf].ap()])
                    za = hd.tile([32, D], F32, tag=f"za{hf}",
                                 name=f"za{hf}")
                    nc.sync.dma_start(za[:], ccz_out[hf].ap())
                    zar.append(za)
                zfull = hd.tile([32, D], F32)
                nc.vector.tensor_tensor(zfull[:], zar[0][:], zar[1][:],
                                        ALU.add)
                nc.vector.tensor_tensor(zfull[:], zfull[:], zb[:], ALU.add)
                zr = hd.tile([32, D], F32)
                nc.scalar.activation(zr[:], zfull[:], ACT.Relu)
                if dbg:
                    nc.sync.dma_start(z_dbg.ap(), zr[:])

                f2w = hd.tile([128, 4, C], F16)
                nc.sync.dma_start(
                    f2w[:], fc2w_in.ap().rearrange("(t p) c -> p t c", p=128))
                lps = ps3.tile([32, C], F32, tag="lg")
                for t4 in range(4):
                    ztp = ps3.tile([128, 32], F32, tag="zt", name=f"zt{t4}")
                    nc.tensor.transpose(ztp[:], zr[:, 128 * t4:128 * (t4 + 1)],
                                        identf[:])
                    zts = hdt.tile([128, 32], F16, tag="zts", name=f"zts{t4}")
                    nc.any.tensor_copy(out=zts[:], in_=ztp[:])
                    nc.tensor.matmul(lps[:], zts[:], f2w[:, t4, :],
                                     start=(t4 == 0), stop=(t4 == 3))
                logits = hd.tile([32, C], F32)
                f2b = hd.tile([32, C], F32)
                nc.sync.dma_start(f2b[:], fc2b_in.ap())
                nc.vector.tensor_tensor(logits[:], lps[:], f2b[:], ALU.add)

                mx = hd.tile([32, 1], F32)
                nc.vector.tensor_reduce(mx[:], logits[:], axis=AX.X, op=ALU.max)
                sh = hd.tile([32, C], F32)
                nc.vector.tensor_tensor(sh[:], logits[:],
                                        mx[:].to_broadcast((32, C)),
                                        ALU.subtract)
                ex = hd.tile([32, C], F32)
                nc.scalar.activation(ex[:], sh[:], ACT.Exp)
                sm = hd.tile([32, 1], F32)
                nc.vector.tensor_reduce(sm[:], ex[:], axis=AX.X, op=ALU.add)
                lg = hd.tile([32, 1], F32)
                nc.scalar.activation(lg[:], sm[:], ACT.Ln)
                res = hd.tile([32, C], F32)
                nc.vector.tensor_tensor(res[:], sh[:],
                                        lg[:].to_broadcast((32, C)),
                                        ALU.subtract)
                nc.sync.dma_start(out_t.ap(), res[:])

    nc.compile()
    return nc


def make_inputs(x, edge_index0, edge_index2, W1, b1, W2, b2,
                fc1_w, fc1_b, fc2_w, fc2_b):
    """Build the 8 per-core input maps."""
    A0 = _dense_adj(np.asarray(edge_index0), N0)
    A2 = _dense_adj(np.asarray(edge_index2), N2)
    M1T = _f16((4.0 * (A0 @ A0)).T)        # [N0, N0], col-sliced per core
    A1T = _f16((2.0 * A0).T)
    A2T = _f16((2.0 * A2).T)
    M2T = _f16((4.0 * (A2 @ A2)).T)

    # rank-16 cosine basis and folded W1
    t = np.arange(T)
    C16 = np.cos(2.0 * np.pi * np.outer(t, np.arange(NF)) / T).astype(np.float32)
    W1f = np.asarray(W1, np.float32)       # [K, T, G1]
    Wf = np.zeros((K, NF, G1), np.float32)
    Wf[:, 0] = W1f[:, 0]
    Wf[:, 15] = W1f[:, 15]
    for fp in range(1, 15):
        Wf[:, fp] = W1f[:, fp] + W1f[:, T - fp]

    # c16 stationary [128 rows (bl2 4, t 32), bb2 2, 128 cols (b_loc 8, f 16)]
    c16w = np.zeros((128, 2, 128), np.float32)
    for bb2 in range(2):
        for bl2 in range(4):
            b_loc = 4 * bb2 + bl2
            c16w[32 * bl2:32 * bl2 + T, bb2,
                 16 * b_loc:16 * (b_loc + 1)] = C16
    c16w = _f16(c16w.reshape(128, 256))

    # w1a stationary [128 rows (b_loc 8, f 16), K, bb 2, 128 cols (bl2, g)]
    w1a = np.zeros((128, K, 2, 128), np.float32)
    for bb in range(2):
        for bl2 in range(4):
            b_loc = 4 * bb + bl2
            w1a[16 * b_loc:16 * (b_loc + 1), :, bb,
                32 * bl2:32 * (bl2 + 1)] = Wf.transpose(1, 0, 2)
    w1a = _f16(w1a.reshape(128, K * 2 * 128))

    # w2a stationary [128 rows (bl2 4, g1 32), K, hh 2, 128 cols (blh 2, g2)]
    W2f = np.asarray(W2, np.float32)       # [K, G1, G2]
    w2a = np.zeros((128, K, 2, 128), np.float32)
    for hh in range(2):
        for blh in range(2):
            bl2 = 2 * hh + blh
            w2a[32 * bl2:32 * (bl2 + 1), :, hh,
                64 * blh:64 * (blh + 1)] = W2f.transpose(1, 0, 2)
    w2a = _f16(w2a.reshape(128, K * 2 * 128))

    b1v = np.tile(np.asarray(b1, np.float32), 4).reshape(128, 1)
    b2v = np.tile(np.asarray(b2, np.float32), 2).reshape(128, 1)
    fc1b = np.tile(np.asarray(fc1_b, np.float32)[None, :], (B, 1))
    fc2b = np.tile(np.asarray(fc2_b, np.float32)[None, :], (B, 1))
    fc2w = _f16(np.asarray(fc2_w, np.float32))
    fc1wf = np.asarray(fc1_w, np.float32)
    xf = np.asarray(x, np.float32)         # [B, N0, T]

    ins = []
    for core in range(NCORES):
        h, q = core // 4, core % 4
        # x_cm [128 rows (bl2 4, t 32), bb2 2, 2048 nodes of half h]
        x_cm = np.zeros((128, 2, NH), np.float32)
        for bb2 in range(2):
            for bl2 in range(4):
                bg = 4 * (4 * bb2 + bl2) + q
                x_cm[32 * bl2:32 * bl2 + T, bb2, :] = \
                    xf[bg, NH * h:NH * (h + 1), :].T
        ins.append({
            "x_cm": _f16(x_cm.reshape(128, 2 * NH)),
            "c16w": c16w,
            "m1t": np.ascontiguousarray(M1T[:, NH * h:NH * (h + 1)]),
            "a1t": np.ascontiguousarray(A1T[:, NH * h:NH * (h + 1)]),
            "a2t": A2T, "m2t": M2T,
            "w1a": w1a, "w2a": w2a, "b1v": b1v, "b2v": b2v,
            "fc1w": _f16(fc1wf[FBLK * core:FBLK * (core + 1), :]),
            "fc1b": fc1b, "fc2b": fc2b, "fc2w": fc2w,
        })
    return ins


def batch_perm():
    """flat row order (r, b_loc) -> global batch id."""
    perm = []
    for r in range(NCORES):
        for bl in range(4):
            perm.append(4 * r + bl)
    return np.array(perm)


_CACHED = {}


def kernel(**inputs):
    if "nc" not in _CACHED:
        _CACHED["nc"] = build_program(dbg=False)
    nc = _CACHED["nc"]
    ins = make_inputs(**inputs)
    res = run_bass_kernel_spmd(nc, ins, core_ids=list(range(NCORES)))
    out = np.zeros((B, C), np.float32)
    out[batch_perm()] = res.results[0]["out"]
    return out


# revision 22
# speedup vs baseline: 1.0777x; 1.0777x over previous
"""NetTGCN forward pass on 8 Trainium2 NeuronCores (Bass/Tile).

Key algorithmic move: the reference's real(FFT) along the 30 time taps is a
rank-16 linear map (cos(2*pi*t*f/30) has identical columns for f and 30-f),
so layer 1's Chebyshev recurrence runs on 16 frequency channels per batch
instead of 30 taps - half the spmv FLOPs of a direct fold.

Sharding:
  Layer 1 (4096-node graph): 2-way node-shard x 4-way batch-shard. Per core:
  8 batches x 16 freqs = 128 channels, 2048 own nodes. The state is kept
  CHANNEL-major [128 c, 2048 n]; the spmv is out = state_blk.T @ M^T-rows
  (stationary = node-major state blocks from the gathered DRAM copy, moving =
  SBUF-resident M^T shard, N=512), which directly produces the channel-major
  next state, so the per-k W-contraction needs no transposes. The per-step
  exchange is a 2-rank AllGather (pairs (c, c+4)) of the XBAR-DMA-transposed
  fp16 state (0.5 MB wire, ~16 us), hidden under the other Chebyshev chain's
  spmv (even/odd chains via M = 4*A'^2). fp16 everywhere in layer 1 (states
  included): simulated end-to-end error 1.8e-3.
  Core (h, q) = core h*4+q owns node half h and batches b_loc -> global
  batch 4*b_loc + q; L2 core j owns batches 4j..4j+3.
  Layer 2 (1024-node graph): batch-parallel (core j handles batches
  4j..4j+3 after an 8-rank AllToAll), zero collectives in the loop,
  same channel-major spmv structure, A2 resident, fc1w prefetched meanwhile.
  Head: h2 features redistributed with an 8-rank AllToAll so fc1 is sharded
  over its 65536-row contraction; partial z AllReduced; fc2 + log_softmax
  computed redundantly on every core. Host un-permutes the 32 rows.
"""

import sys

if "/opt/trn_rl_repo" not in sys.path:
    sys.path.insert(0, "/opt/trn_rl_repo")

import numpy as np

import concourse.bacc as bacc
import concourse.mybir as mybir
import concourse.bass_utils as _bu
from concourse.bass_utils import run_bass_kernel_spmd
from concourse.tile import TileContext
from concourse.masks import make_identity

_bu.upload_artifacts = lambda tmpdir: f"file://{tmpdir}"  # no bucket in sandbox

F16 = mybir.dt.float16
F32 = mybir.dt.float32
AX = mybir.AxisListType
ALU = mybir.AluOpType
ACT = mybir.ActivationFunctionType

B, N0, T, K = 32, 4096, 30, 25
G1, G2, D, C = 32, 64, 512, 10
N2 = N0 // 4
NF = 16                 # rank of the real-FFT cosine map
NCORES = 8
NH = N0 // 2            # 2048 own nodes per core (node half)
P2H = N2 // 2           # 512 own pooled nodes
FBLK = (N2 * G2) // NCORES  # 8192 fc1 contraction rows per core

GPAIR = [[0, 4], [1, 5], [2, 6], [3, 7]]
G8 = [list(range(NCORES))]


def _f16(a):
    return np.ascontiguousarray(np.asarray(a, np.float32).astype(np.float16))


def _dense_adj(edge_index, n):
    row = edge_index[0].astype(np.int64)
    col = edge_index[1].astype(np.int64)
    deg = np.zeros(n, np.float32)
    np.add.at(deg, row, 1.0)
    dis = np.where(deg > 0, 1.0 / np.sqrt(np.maximum(deg, 1.0)), 0.0).astype(np.float32)
    w = (-dis[row] * dis[col]).astype(np.float32)
    a = np.zeros((n, n), np.float32)
    np.add.at(a, (row, col), w)
    return a


def build_program(dbg=False):
    nc = bacc.Bacc("TRN2", target_bir_lowering=False, debug=False,
                   num_devices=NCORES)

    x_cm_in = nc.dram_tensor("x_cm", [128, 2 * NH], F16, kind="ExternalInput")
    c16_in = nc.dram_tensor("c16w", [128, 2 * 128], F16, kind="ExternalInput")
    m1t_in = nc.dram_tensor("m1t", [N0, NH], F16, kind="ExternalInput")
    a1t_in = nc.dram_tensor("a1t", [N0, NH], F16, kind="ExternalInput")
    a2t_in = nc.dram_tensor("a2t", [N2, N2], F16, kind="ExternalInput")
    m2t_in = nc.dram_tensor("m2t", [N2, N2], F16, kind="ExternalInput")
    w1_in = nc.dram_tensor("w1a", [128, K * 2 * 128], F16, kind="ExternalInput")
    w2_in = nc.dram_tensor("w2a", [128, K * 2 * 128], F16, kind="ExternalInput")
    b1_in = nc.dram_tensor("b1v", [128, 1], F32, kind="ExternalInput")
    b2_in = nc.dram_tensor("b2v", [128, 1], F32, kind="ExternalInput")
    fc1w_in = nc.dram_tensor("fc1w", [FBLK, D], F16, kind="ExternalInput")
    fc1b_in = nc.dram_tensor("fc1b", [B, D], F32, kind="ExternalInput")
    fc2w_in = nc.dram_tensor("fc2w", [D, C], F16, kind="ExternalInput")
    fc2b_in = nc.dram_tensor("fc2b", [B, C], F32, kind="ExternalInput")

    out_t = nc.dram_tensor("out", [B, C], F32, kind="ExternalOutput")
    if dbg:
        h1_dbg = nc.dram_tensor("h1_dbg", [256, NH], F32, kind="ExternalOutput")
        l2i_dbg = nc.dram_tensor("l2i_dbg", [128, N2], F32, kind="ExternalOutput")
        h2_dbg = nc.dram_tensor("h2_dbg", [256, N2], F32, kind="ExternalOutput")
        ccpo_dbg = nc.dram_tensor("ccpo_dbg", [256, P2H], F16,
                                  kind="ExternalOutput")
        nm0_dbg = nc.dram_tensor("nm0_dbg", [N2, 128], F16,
                                 kind="ExternalOutput")
        t22_dbg = nc.dram_tensor("t22_dbg", [128, N2], F32,
                                 kind="ExternalOutput")
        z_dbg = nc.dram_tensor("z_dbg", [B, D], F32, kind="ExternalOutput")

    cc1_in = [nc.dram_tensor(f"cc1i{i}", [NH, 128], F16) for i in range(2)]
    cc1_out = [nc.dram_tensor(f"cc1o{i}", [N0, 128], F16) for i in range(2)]
    ccp_in = nc.dram_tensor("ccp_in", [256, P2H], F16)
    ccp_out = nc.dram_tensor("ccp_out", [256, P2H], F16)
    cch_in = nc.dram_tensor("cch_in", [NCORES * 4, FBLK], F16)
    cch_out = nc.dram_tensor("cch_out", [NCORES * 4, FBLK], F16)
    ccz_in = nc.dram_tensor("ccz_in", [B, D], F32)
    ccz_out = nc.dram_tensor("ccz_out", [B, D], F32, addr_space="Shared")

    with TileContext(nc) as tc:
        # ======================= LAYER 1 =======================
        with tc.tile_pool(name="l1c", bufs=1) as l1c:
          with tc.tile_pool(name="l1mv", bufs=1) as l1mv, \
             tc.tile_pool(name="l1st", bufs=5) as l1st, \
             tc.tile_pool(name="l1g", bufs=3) as l1g, \
             tc.tile_pool(name="l1nm", bufs=2) as l1nm, \
             tc.tile_pool(name="ps_y", bufs=3, space="PSUM") as ps_y, \
             tc.tile_pool(name="ps_ct", bufs=2, space="PSUM") as ps_ct:

            w1a = l1c.tile([128, K, 2, 128], F16)
            nc.sync.dma_start(
                w1a[:], w1_in.ap().rearrange("p (k b c) -> p k b c", k=K, b=2))
            b1v = l1c.tile([128, 1], F32)
            nc.sync.dma_start(b1v[:], b1_in.ap())
            h1_sb = l1c.tile([128, 2, NH], F32)
            nc.any.memset(h1_sb[:], 0.0)

            # moving-operand buffer: holds a1t for k=1, then m1t for k>=2.
            # Bulk loads ride the scalar-engine HWDGE queue so the
            # latency-critical sync-queue DMAs are not stuck behind them.
            mv = l1mv.tile([128, 32, NH], F16)
            a1_v = a1t_in.ap().rearrange("(t p) n -> t p n", p=128)
            m1_v = m1t_in.ap().rearrange("(t p) n -> t p n", p=128)
            for mb in range(32):
                nc.scalar.dma_start(mv[:, mb, :], a1_v[mb])

            tx = {}

            def xbar_ag(k):
                nm = l1nm.tile([128, NH // 128, 128], F16, tag="nm",
                               name=f"nm{k}")
                nc.sync.dma_start_transpose(nm[:], tx[k][:])
                cin, cout = cc1_in[k % 2], cc1_out[k % 2]
                nc.sync.dma_start(
                    cin.ap().rearrange("(t p) c -> p t c", p=128), nm[:])
                nc.gpsimd.collective_compute(
                    "AllGather", ALU.bypass, replica_groups=GPAIR,
                    ins=[cin.ap()], outs=[cout.ap()])

            def contract(k):
                for bb in range(2):
                    for ns in range(4):
                        cps = ps_ct.tile([128, 512], F32, tag="ct",
                                         name=f"ct{k}_{bb}_{ns}")
                        nc.tensor.matmul(cps[:], w1a[:, k, bb, :],
                                         tx[k][:, 512 * ns:512 * (ns + 1)],
                                         start=True, stop=True)
                        nc.vector.tensor_tensor(
                            h1_sb[:, bb, 512 * ns:512 * (ns + 1)],
                            h1_sb[:, bb, 512 * ns:512 * (ns + 1)],
                            cps[:], ALU.add)

            # ---- x' = x @ C16 (channel-major) ----
            with tc.tile_pool(name="l1x", bufs=1) as l1x:
                c16 = l1x.tile([128, 2, 128], F16)
                nc.sync.dma_start(
                    c16[:], c16_in.ap().rearrange("p (b c) -> p b c", b=2))
                x_v = x_cm_in.ap().rearrange("p (b n) -> p b n", b=2)
                tx[0] = l1st.tile([128, NH], F16, tag="tx", name="tx0")
                for bb2 in range(2):
                    xh = l1x.tile([128, NH], F16, tag="xh", name=f"xh{bb2}")
                    nc.sync.dma_start(xh[:], x_v[:, bb2, :])
                    for ns in range(4):
                        xps = ps_ct.tile([128, 512], F32, tag="ct",
                                         name=f"xp{bb2}_{ns}")
                        nc.tensor.matmul(xps[:], c16[:, bb2, :],
                                         xh[:, 512 * ns:512 * (ns + 1)],
                                         start=True, stop=True)
                        o = tx[0][:, 512 * ns:512 * (ns + 1)]
                        if bb2 == 0:
                            nc.vector.tensor_copy(o, xps[:])
                        else:
                            nc.vector.tensor_tensor(o, o, xps[:], ALU.add)
                xbar_ag(0)

            # ---- Chebyshev steps; contract(k-1) emitted between spmvs ----
            for k in range(1, K):
                gi = 0 if k == 1 else k % 2   # k=1 consumes the x' gather
                gsrc = cc1_out[gi].ap().rearrange("(t p) c -> p t c", p=128)
                tx[k] = l1st.tile([128, NH], F16, tag="tx", name=f"tx{k}")
                stt = []
                for hb in range(2):
                    s = l1g.tile([128, 16, 128], F16, tag="g",
                                 name=f"g{k}_{hb}")
                    nc.sync.dma_start(s[:], gsrc[:, 16 * hb:16 * (hb + 1), :])
                    stt.append(s)
                for half in range(2):
                    yp = ps_y.tile([128, 2, 512], F32, tag="y",
                                   name=f"y{k}_{half}")
                    for mb in range(32):
                        for j in range(2):
                            nc.tensor.matmul(
                                yp[:, j, :], stt[mb // 16][:, mb % 16, :],
                                mv[:, mb, 1024 * half + 512 * j:
                                   1024 * half + 512 * (j + 1)],
                                start=(mb == 0), stop=(mb == 31))
                    o = tx[k][:, 1024 * half:1024 * (half + 1)]
                    ypf = yp[:].rearrange("p a b -> p (a b)")
                    if k == 1:
                        nc.vector.tensor_scalar_mul(o, ypf, 0.5)
                    elif k == 2:
                        nc.vector.tensor_scalar_mul(o, ypf, 0.5)
                        nc.vector.tensor_tensor(
                            o, o, tx[0][:, 1024 * half:1024 * (half + 1)],
                            ALU.subtract)
                    elif k == 3:
                        p1 = tx[1][:, 1024 * half:1024 * (half + 1)]
                        nc.vector.tensor_tensor(o, ypf, p1, ALU.subtract)
                        nc.vector.tensor_tensor(o, o, p1, ALU.subtract)
                        nc.vector.tensor_tensor(o, o, p1, ALU.subtract)
                    else:
                        p2 = tx[k - 2][:, 1024 * half:1024 * (half + 1)]
                        p4 = tx[k - 4][:, 1024 * half:1024 * (half + 1)]
                        nc.vector.tensor_tensor(o, ypf, p2, ALU.subtract)
                        nc.vector.tensor_tensor(o, o, p2, ALU.subtract)
                        nc.vector.tensor_tensor(o, o, p4, ALU.subtract)
                if k == 1:
                    # refill the moving buffer with m1t now that a1t is done
                    for mb in range(32):
                        nc.scalar.dma_start(mv[:, mb, :], m1_v[mb])
                if k < K - 2:
                    xbar_ag(k)
                contract(k - 1)
                tx.pop(k - 4, None)
            contract(K - 1)

          # inner pools close here so layer 2's bulk constant loads can
          # reuse their SBUF while the pool/bias/AllToAll tail runs.
          if True:
            # ---- bias + relu + maxpool4 along nodes ----
            h1p = l1c.tile([128, 2, P2H], F16)
            for bb in range(2):
                nc.scalar.activation(h1_sb[:, bb, :], h1_sb[:, bb, :],
                                     ACT.Relu, bias=b1v[:])
                h4 = h1_sb[:, bb, :].rearrange("p (n f) -> p n f", f=4)
                nc.vector.tensor_tensor(h1p[:, bb, :], h4[:, :, 0],
                                        h4[:, :, 1], ALU.max)
                nc.vector.tensor_tensor(h1p[:, bb, :], h1p[:, bb, :],
                                        h4[:, :, 2], ALU.max)
                nc.vector.tensor_tensor(h1p[:, bb, :], h1p[:, bb, :],
                                        h4[:, :, 3], ALU.max)
            if dbg:
                nc.sync.dma_start(
                    h1_dbg.ap().rearrange("(b p) n -> p b n", p=128), h1_sb[:])

            # 8-rank AllToAll of pooled features. Batch ownership is chosen so
            # slot j (rows 32j..32j+32 = b_loc j's g-rows x own 512 nodes) is
            # exactly what L2 core j needs from this core; the output blocks
            # are then read rank-uniformly.
            nc.sync.dma_start(
                ccp_in.ap().rearrange("(b p) c -> p b c", p=128), h1p[:])
            nc.gpsimd.collective_compute(
                "AllToAll", ALU.bypass, replica_groups=G8,
                ins=[ccp_in.ap()], outs=[ccp_out.ap()])

        # ======================= LAYER 2 =======================
        with tc.tile_pool(name="l2c", bufs=1) as l2c, \
             tc.tile_pool(name="l2st", bufs=5) as l2st, \
             tc.tile_pool(name="l2nm", bufs=3) as l2nm:

            a2t = l2c.tile([128, N2 // 128, N2], F16)
            nc.scalar.dma_start(
                a2t[:], a2t_in.ap().rearrange("(t p) n -> p t n", p=128))
            m2t = l2c.tile([128, N2 // 128, N2], F16)
            nc.scalar.dma_start(
                m2t[:], m2t_in.ap().rearrange("(t p) n -> p t n", p=128))
            w2a = l2c.tile([128, K, 2, 128], F16)
            nc.scalar.dma_start(
                w2a[:], w2_in.ap().rearrange("p (k h c) -> p k h c", k=K, h=2))
            b2v = l2c.tile([128, 1], F32)
            nc.sync.dma_start(b2v[:], b2_in.ap())
            # preload fc1w for the head while layer 2 computes
            fc1w = l2c.tile([128, FBLK // 128, D], F16)
            nc.scalar.dma_start(
                fc1w[:], fc1w_in.ap().rearrange("(t p) d -> p t d", p=128))
            h2_sb = l2c.tile([128, 2, N2], F32)
            nc.any.memset(h2_sb[:], 0.0)

            ident2 = l2c.tile([128, 128], F16)
            make_identity(nc, ident2[:])
            with tc.tile_pool(name="ps2_y", bufs=2, space="PSUM") as ps2_y, \
                 tc.tile_pool(name="ps2_ct", bufs=2, space="PSUM") as ps2_ct, \
                 tc.tile_pool(name="ps2_tr", bufs=2, space="PSUM") as ps2_tr:

                tx2 = {}
                nm2 = {}
                # out block r=(h', q') = core r's slot for me: batch 4*my_j+q'
                # (g1-rows) x n2-half h'
                tx2[0] = l2st.tile([128, N2], F16, tag="tx2", name="tx20")
                for hp in range(2):
                    for qp in range(4):
                        nc.sync.dma_start(
                            tx2[0][32 * qp:32 * (qp + 1),
                                   512 * hp:512 * (hp + 1)],
                            ccp_out.ap()[32 * (4 * hp + qp):
                                         32 * (4 * hp + qp + 1), :])
                if dbg:
                    l2i = l2c.tile([128, N2], F32)
                    nc.vector.tensor_copy(l2i[:], tx2[0][:])
                    nc.sync.dma_start(l2i_dbg.ap(), l2i[:])
                    ccst = l2c.tile([128, 2, P2H], F16)
                    nc.sync.dma_start(
                        ccst[:],
                        ccp_out.ap().rearrange("(a p) c -> p a c", p=128))
                    nc.sync.dma_start(
                        ccpo_dbg.ap().rearrange("(a p) c -> p a c", p=128),
                        ccst[:])

                def xbar2(k):
                    # PE transposes (XBAR->PE edges proved racy on HW)
                    nm2[k] = l2nm.tile([128, N2 // 128, 128], F16, tag="nm2",
                                       name=f"nm2_{k}")
                    for g4 in range(2):
                        trp = ps2_tr.tile([128, 4, 128], F16, tag="tr2",
                                          name=f"tr2_{k}_{g4}")
                        for t in range(4):
                            mb = 4 * g4 + t
                            nc.tensor.transpose(
                                trp[:, t, :],
                                tx2[k][:, 128 * mb:128 * (mb + 1)],
                                ident2[:])
                            nc.any.tensor_copy(out=nm2[k][:, mb, :],
                                               in_=trp[:, t, :])

                def contract2(k):
                    for hh in range(2):
                        for ns in range(2):
                            cps = ps2_ct.tile([128, 512], F32, tag="ct2",
                                              name=f"c2_{k}_{hh}_{ns}")
                            nc.tensor.matmul(
                                cps[:], w2a[:, k, hh, :],
                                tx2[k][:, 512 * ns:512 * (ns + 1)],
                                start=True, stop=True)
                            nc.vector.tensor_tensor(
                                h2_sb[:, hh, 512 * ns:512 * (ns + 1)],
                                h2_sb[:, hh, 512 * ns:512 * (ns + 1)],
                                cps[:], ALU.add)

                # even/odd chains via M2 = 4*A2'^2 (same scheme as layer 1):
                # spmv k consumes nm2[k-2], so the update/XBAR latency of a
                # step hides under the other chain's spmv.
                xbar2(0)
                for k in range(1, K):
                    tx2[k] = l2st.tile([128, N2], F16, tag="tx2",
                                       name=f"tx2{k}")
                    src_nm = nm2[0] if k <= 2 else nm2[k - 2]
                    mvop = a2t if k == 1 else m2t
                    yp = ps2_y.tile([128, 2, 512], F32, tag="y2",
                                    name=f"y2_{k}")
                    for mb in range(N2 // 128):
                        for j in range(2):
                            nc.tensor.matmul(
                                yp[:, j, :], src_nm[:, mb, :],
                                mvop[:, mb, 512 * j:512 * (j + 1)],
                                start=(mb == 0), stop=(mb == N2 // 128 - 1))
                    ypf = yp[:].rearrange("p a b -> p (a b)")
                    if k == 1:
                        nc.vector.tensor_scalar_mul(tx2[1][:], ypf, 0.5)
                    elif k == 2:
                        nc.vector.tensor_scalar_mul(tx2[2][:], ypf, 0.5)
                        nc.vector.tensor_tensor(tx2[2][:], tx2[2][:],
                                                tx2[0][:], ALU.subtract)
                    elif k == 3:
                        nc.vector.tensor_tensor(tx2[3][:], ypf, tx2[1][:],
                                                ALU.subtract)
                        nc.vector.tensor_tensor(tx2[3][:], tx2[3][:],
                                                tx2[1][:], ALU.subtract)
                        nc.vector.tensor_tensor(tx2[3][:], tx2[3][:],
                                                tx2[1][:], ALU.subtract)
                    else:
                        nc.vector.tensor_tensor(tx2[k][:], ypf,
                                                tx2[k - 2][:], ALU.subtract)
                        nc.vector.tensor_tensor(tx2[k][:], tx2[k][:],
                                                tx2[k - 2][:], ALU.subtract)
                        nc.vector.tensor_tensor(tx2[k][:], tx2[k][:],
                                                tx2[k - 4][:], ALU.subtract)
                    if k < K - 2:
                        xbar2(k)
                    contract2(k - 1)
                    if dbg and k == 2:
                        t22 = l2c.tile([128, N2], F32, name="t22")
                        nc.vector.tensor_copy(t22[:], tx2[2][:])
                        nc.sync.dma_start(t22_dbg.ap(), t22[:])
                        nc.sync.dma_start(
                            nm0_dbg.ap().rearrange("(t p) c -> p t c", p=128),
                            nm2[0][:])
                    nm2.pop(k - 4, None)
                    tx2.pop(k - 4, None)
                contract2(K - 1)

                # bias + relu -> fp16 channel-major h2
                h2r = l2c.tile([128, 2, N2], F16)
                for hh in range(2):
                    nc.scalar.activation(h2r[:, hh, :], h2_sb[:, hh, :],
                                         ACT.Relu, bias=b2v[:])
                if dbg:
                    h2f = l2c.tile([128, 2, N2], F32)
                    nc.vector.tensor_copy(h2f[:], h2r[:])
                    nc.sync.dma_start(
                        h2_dbg.ap().rearrange("(h p) n -> p h n", p=128),
                        h2f[:])

            # =================== HEAD ===================
            with tc.tile_pool(name="hd", bufs=1) as hd, \
                 tc.tile_pool(name="hdt", bufs=2) as hdt, \
                 tc.tile_pool(name="ps3", bufs=2, space="PSUM") as ps3, \
                 tc.tile_pool(name="ps3z", bufs=1, space="PSUM") as ps3z:

                ident = hd.tile([128, 128], F16)
                make_identity(nc, ident[:])
                identf = hd.tile([32, 32], F32)
                make_identity(nc, identf[:])
                # ft[n2p, nt, (b4, g64)] fp16 via PE transposes
                ft = hd.tile([128, N2 // 128, 256], F16)
                for hh in range(2):
                    for nt in range(N2 // 128):
                        tmp = ps3.tile([128, 128], F16, tag="zt",
                                       name=f"t3_{hh}_{nt}")
                        nc.tensor.transpose(
                            tmp[:], h2r[:, hh, 128 * nt:128 * (nt + 1)],
                            ident[:])
                        for blh in range(2):
                            nc.any.tensor_copy(
                                out=ft[:, nt,
                                       64 * (2 * hh + blh):
                                       64 * (2 * hh + blh + 1)],
                                in_=tmp[:, 64 * blh:64 * (blh + 1)])
                # cch_in rows (r 8, b 4), cols f = (n2p 128, g 64)
                cch_v = cch_in.ap().rearrange("(r b) (p g) -> r p b g",
                                              b=4, p=128)
                for r in range(N2 // 128):
                    nc.sync.dma_start(
                        cch_v[r],
                        ft[:, r, :].rearrange("p (b g) -> p b g", b=4))
                nc.gpsimd.collective_compute(
                    "AllToAll", ALU.bypass, replica_groups=G8,
                    ins=[cch_in.ap()], outs=[cch_out.ap()])

                # fc1: flt[f-part, kt, 32 rb] via XBAR from cch_out; the
                # XBAR lands in a staging tile and a same-queue DMA copies it,
                # so the PE consumer sees a regular DMA-write edge.
                flt_raw = hd.tile([128, FBLK // 128, B], F16)
                nc.sync.dma_start_transpose(flt_raw[:], cch_out.ap())
                flt = hd.tile([128, FBLK // 128, B], F16)
                nc.sync.dma_start(flt[:], flt_raw[:])
                zps = ps3z.tile([32, D], F32)
                for kt in range(FBLK // 128):
                    nc.tensor.matmul(zps[:], flt[:, kt, :], fc1w[:, kt, :],
                                     start=(kt == 0),
                                     stop=(kt == FBLK // 128 - 1))
                zblk = hd.tile([32, D], F32)
                nc.vector.tensor_copy(zblk[:], zps[:])
                nc.sync.dma_start(ccz_in.ap(), zblk[:])
                nc.gpsimd.collective_compute(
                    "AllReduce", ALU.add, replica_groups=G8,
                    ins=[ccz_in.ap()], outs=[ccz_out.ap()])
                zfull = hd.tile([32, D], F32)
                nc.sync.dma_start(zfull[:], ccz_out.ap())
                zb = hd.tile([32, D], F32)
                nc.sync.dma_start(zb[:], fc1b_in.ap())
                nc.vector.tensor_tensor(zfull[:], zfull[:], zb[:], ALU.add)
                zr = hd.tile([32, D], F32)
                nc.scalar.activation(zr[:], zfull[:], ACT.Relu)
                if dbg:
                    nc.sync.dma_start(z_dbg.ap(), zr[:])

                f2w = hd.tile([128, 4, C], F16)
                nc.sync.dma_start(
                    f2w[:], fc2w_in.ap().rearrange("(t p) c -> p t c", p=128))
                lps = ps3.tile([32, C], F32, tag="lg")
                for t4 in range(4):
                    ztp = ps3.tile([128, 32], F32, tag="zt", name=f"zt{t4}")
                    nc.tensor.transpose(ztp[:], zr[:, 128 * t4:128 * (t4 + 1)],
                                        identf[:])
                    zts = hdt.tile([128, 32], F16, tag="zts", name=f"zts{t4}")
                    nc.any.tensor_copy(out=zts[:], in_=ztp[:])
                    nc.tensor.matmul(lps[:], zts[:], f2w[:, t4, :],
                                     start=(t4 == 0), stop=(t4 == 3))
                logits = hd.tile([32, C], F32)
                f2b = hd.tile([32, C], F32)
                nc.sync.dma_start(f2b[:], fc2b_in.ap())
                nc.vector.tensor_tensor(logits[:], lps[:], f2b[:], ALU.add)

                mx = hd.tile([32, 1], F32)
                nc.vector.tensor_reduce(mx[:], logits[:], axis=AX.X, op=ALU.max)
                sh = hd.tile([32, C], F32)
                nc.vector.tensor_tensor(sh[:], logits[:],
                                        mx[:].to_broadcast((32, C)),
                                        ALU.subtract)
                ex = hd.tile([32, C], F32)
                nc.scalar.activation(ex[:], sh[:], ACT.Exp)
                sm = hd.tile([32, 1], F32)
                nc.vector.tensor_reduce(sm[:], ex[:], axis=AX.X, op=ALU.add)
                lg = hd.tile([32, 1], F32)
                nc.scalar.activation(lg[:], sm[:], ACT.Ln)
                res = hd.tile([32, C], F32)
                nc.vector.tensor_tensor(res[:], sh[:],
                                        lg[:].to_broadcast((32, C)),
                                        ALU.subtract)
                nc.sync.dma_start(out_t.ap(), res[:])

    nc.compile()
    return nc


def make_inputs(x, edge_index0, edge_index2, W1, b1, W2, b2,
                fc1_w, fc1_b, fc2_w, fc2_b):
    """Build the 8 per-core input maps."""
    A0 = _dense_adj(np.asarray(edge_index0), N0)
    A2 = _dense_adj(np.asarray(edge_index2), N2)
    M1T = _f16((4.0 * (A0 @ A0)).T)        # [N0, N0], col-sliced per core
    A1T = _f16((2.0 * A0).T)
    A2T = _f16((2.0 * A2).T)
    M2T = _f16((4.0 * (A2 @ A2)).T)

    # rank-16 cosine basis and folded W1
    t = np.arange(T)
    C16 = np.cos(2.0 * np.pi * np.outer(t, np.arange(NF)) / T).astype(np.float32)
    W1f = np.asarray(W1, np.float32)       # [K, T, G1]
    Wf = np.zeros((K, NF, G1), np.float32)
    Wf[:, 0] = W1f[:, 0]
    Wf[:, 15] = W1f[:, 15]
    for fp in range(1, 15):
        Wf[:, fp] = W1f[:, fp] + W1f[:, T - fp]

    # c16 stationary [128 rows (bl2 4, t 32), bb2 2, 128 cols (b_loc 8, f 16)]
    c16w = np.zeros((128, 2, 128), np.float32)
    for bb2 in range(2):
        for bl2 in range(4):
            b_loc = 4 * bb2 + bl2
            c16w[32 * bl2:32 * bl2 + T, bb2,
                 16 * b_loc:16 * (b_loc + 1)] = C16
    c16w = _f16(c16w.reshape(128, 256))

    # w1a stationary [128 rows (b_loc 8, f 16), K, bb 2, 128 cols (bl2, g)]
    w1a = np.zeros((128, K, 2, 128), np.float32)
    for bb in range(2):
        for bl2 in range(4):
            b_loc = 4 * bb + bl2
            w1a[16 * b_loc:16 * (b_loc + 1), :, bb,
                32 * bl2:32 * (bl2 + 1)] = Wf.transpose(1, 0, 2)
    w1a = _f16(w1a.reshape(128, K * 2 * 128))

    # w2a stationary [128 rows (bl2 4, g1 32), K, hh 2, 128 cols (blh 2, g2)]
    W2f = np.asarray(W2, np.float32)       # [K, G1, G2]
    w2a = np.zeros((128, K, 2, 128), np.float32)
    for hh in range(2):
        for blh in range(2):
            bl2 = 2 * hh + blh
            w2a[32 * bl2:32 * (bl2 + 1), :, hh,
                64 * blh:64 * (blh + 1)] = W2f.transpose(1, 0, 2)
    w2a = _f16(w2a.reshape(128, K * 2 * 128))

    b1v = np.tile(np.asarray(b1, np.float32), 4).reshape(128, 1)
    b2v = np.tile(np.asarray(b2, np.float32), 2).reshape(128, 1)
    fc1b = np.tile(np.asarray(fc1_b, np.float32)[None, :], (B, 1))
    fc2b = np.tile(np.asarray(fc2_b, np.float32)[None, :], (B, 1))
    fc2w = _f16(np.asarray(fc2_w, np.float32))
    fc1wf = np.asarray(fc1_w, np.float32)
    xf = np.asarray(x, np.float32)         # [B, N0, T]

    ins = []
    for core in range(NCORES):
        h, q = core // 4, core % 4
        # x_cm [128 rows (bl2 4, t 32), bb2 2, 2048 nodes of half h]
        x_cm = np.zeros((128, 2, NH), np.float32)
        for bb2 in range(2):
            for bl2 in range(4):
                bg = 4 * (4 * bb2 + bl2) + q
                x_cm[32 * bl2:32 * bl2 + T, bb2, :] = \
                    xf[bg, NH * h:NH * (h + 1), :].T
        ins.append({
            "x_cm": _f16(x_cm.reshape(128, 2 * NH)),
            "c16w": c16w,
            "m1t": np.ascontiguousarray(M1T[:, NH * h:NH * (h + 1)]),
            "a1t": np.ascontiguousarray(A1T[:, NH * h:NH * (h + 1)]),
            "a2t": A2T, "m2t": M2T,
            "w1a": w1a, "w2a": w2a, "b1v": b1v, "b2v": b2v,
            "fc1w": _f16(fc1wf[FBLK * core:FBLK * (core + 1), :]),
            "fc1b": fc1b, "fc2b": fc2b, "fc2w": fc2w,
        })
    return ins


def batch_perm():
    """flat row order (r, b_loc) -> global batch id."""
    perm = []
    for r in range(NCORES):
        for bl in range(4):
            perm.append(4 * r + bl)
    return np.array(perm)


_CACHED = {}


def kernel(**inputs):
    if "nc" not in _CACHED:
        _CACHED["nc"] = build_program(dbg=False)
    nc = _CACHED["nc"]
    ins = make_inputs(**inputs)
    res = run_bass_kernel_spmd(nc, ins, core_ids=list(range(NCORES)))
    out = np.zeros((B, C), np.float32)
    out[batch_perm()] = res.results[0]["out"]
    return out
